# revision 58
# baseline (speedup 1.0000x reference)
"""Normalized-adjacency kernel (EstimateAdj.normalize, symmetric=False) for TRN2.

out = mx * r_inv[:, None] * r_inv[None, :]   where mx = adj + I,
r_inv = rowsum(mx) ** -0.5.

Strategy (8 NeuronCores, raw Bass, fp16 data movement), "chunked-AG":
  - host: mx' = (adj + I) * 2^13 cast to fp16; INTERLEAVED row sharding:
    core c owns global rows {t*1024 + c*128 + p}, i.e. tile t of core c is
    the global 128-row band t*1024 + c*128. With this map, "tiles 0-3 of
    every core" = global rows [0, 4096) = a contiguous half of the COLUMNS
    for the later column scale, so the AllGather can be split in two chunks
    that pipeline with the load phase without fragmenting the stores.
  - device, per core (shard 1024 x 8192 fp16 = 16 MiB resident in SBUF):
      8 tile loads [128 x 8192] on the gpsimd SWDGE ring (~335 GB/s).
      rowsum per tile split ACT prefix [0:AC) (Copy + f32 accum - MUST
      start at column 0: a non-zero source offset drops the ACT Copy from
      2x to 1x mode, measured 7.1 us vs 3.3 us) / DVE suffix [AC:) via
      tensor_reduce; AC shrunk for the chunk-closing tiles 3 and 7.
      Per-chunk r_inv' chain: DVE comb (psa+psb) -> ACT sqrt(ps * 2^-26)
      (drain-publish) -> PE transpose [128,4]->[4,128] -> DVE reciprocal
      out of PSUM -> 1 KiB payload DMA on the SYNC ring (NOT the gpsimd
      ring, where it would FIFO behind the remaining 2 MiB tile loads -
      measured +14 us) -> AllGather chunk k (1 KiB/rank).
      Both AG doorbells are gated on loads-done: the ncfw entry phase
      stalls under heavy HBM traffic anyway (ring start is always
      ~loads_end + 13-17 us regardless of doorbell time), and a post-load
      doorbell measures a few us faster to ring-start.  AG0's entry+ring
      finish ~82 us; AG1 serializes on the CC cores right after (~89 us).
      Row scale: tiles 0-3 on DVE (tensor_scalar 4x) interleaved with the
      chunk-1 reduces; tiles 4-7 on ACT (Copy + per-partition scale) in
      the AG1 window, h-half-major, batch drain-published per half (s_ts).
      Column scale: the gathered 8 KiB r_inv row is DMA'd to partition 0,
      broadcast to [128 x 4096] on the PE (ones[1,128].T @ row matmuls
      into fp32 PSUM, 512-col bank pieces) and copied fp32->fp16 to SBUF
      by DVE - the DRE partition-broadcast DMA path measures 8-14 us vs
      ~5 for matmul+copy.  The (c,t,p)->(t,c,p) permutation between AG
      concat order and global column order is folded into the col-scale
      tensor_tensor src1 access pattern (inner 128-elem runs stay
      contiguous so the TT keeps 2x packed mode, 2.28 us per half-tile).
      Stores: 16 x 1 MiB per-tile-half gated per TT - half 0 on the sync
      HWDGE ring (draining while AG1 is still in flight), half 1 on the
      ACT ring (splitting half 1 across both rings measured slower).
  - host: upcast, divide by 2^26, un-interleave rows.
  Measured: baseline single-AG 164.3 us -> this version 145.8-146.6 us
  (rel err 2.13e-3, gate 2e-2).  The remaining fat is the collective:
  ~13-17 us ncfw entry per AG (HBM-traffic-throttled) + ~5-6 us ring,
  x2 serialized, and the 16 x 2.28 us serial DVE TT stream they gate.
  Known-fatal variants: gpsimd (Pool) tensor_tensor / gpsimd
  partition_broadcast DMA -> NRT_EXEC_UNIT_UNRECOVERABLE; activation
  bias=AP reads stale data (use a real DVE combine); DVE tensor_reduce
  outputs need a self-wait + dummy-read drain before any reader (lazy
  accumulation-path writeback); ACT Copy+accum drops 2x->1x mode unless
  the source starts at column offset 0.

(remote_dma peer-write exchange would cut the collective further but
InstRemoteDMA*/hostgen variants fail neuronxcc walrus codegen on this
toolchain: "ISA wrong length" in CoreV2GenImpl visitInstISA.)
"""

from contextlib import ExitStack

import numpy as np

import concourse.bass as bass
import concourse.mybir as mybir
from concourse.bass_utils import run_bass_kernel_spmd

N = 8192
NCORES = 8
SHARD = N // NCORES  # 1024
P = 128
T = SHARD // P  # 8 tiles per core
H = 2  # column halves (chunk granularity), 4096 cols each
CT = T // 2  # tiles per chunk (4)
W = N // H  # 4096
AC = 5440  # ACT rowsum prefix columns (DVE takes the suffix)
ACC = 4864  # smaller ACT prefix for the chunk-closing tiles (3 and 7)

F16 = mybir.dt.float16
F32 = mybir.dt.float32

SCALE_IN = 8192.0  # 2^13
SCALE_OUT = float(2**26)


def build_kernel(n=N, ncores=NCORES):
    shard = n // ncores
    tt = shard // P  # 8
    w = n // H  # 4096
    ct = tt // 2  # 4

    nc = bass.Bass(num_devices=ncores)
    mx = nc.dram_tensor("mx", [shard, n], F16, kind="ExternalInput")
    eye = nc.dram_tensor("eye", [P, P], F16, kind="ExternalInput")
    ones = nc.dram_tensor("ones", [1, P], F16, kind="ExternalInput")
    out = nc.dram_tensor("out", [shard, n], F16, kind="ExternalOutput")
    cc_in = [nc.dram_tensor(f"cc_in{k}", [ct * P], F16) for k in range(H)]
    cc_out = [
        nc.dram_tensor(f"cc_out{k}", [ncores * ct * P], F16, addr_space="Shared")
        for k in range(H)
    ]

    mx_l = mx.rearrange("(t p) m -> t p m", p=P)
    out_v = out.rearrange("(t p) (h w) -> t p h w", p=P, h=H)

    with ExitStack() as ctx:
        tiles = [
            ctx.enter_context(nc.sbuf_tensor(f"tile{t}", [P, n], F16))
            for t in range(tt)
        ]
        colscale = ctx.enter_context(nc.sbuf_tensor("colscale", [P, n], F16))
        eye_sb = ctx.enter_context(nc.sbuf_tensor("eye_sb", [P, P], F16))
        ones_sb = ctx.enter_context(nc.sbuf_tensor("ones_sb", [1, P], F16))
        ccrow = [
            ctx.enter_context(nc.sbuf_tensor(f"ccrow{k}", [1, w], F16))
            for k in range(H)
        ]
        psa = ctx.enter_context(nc.sbuf_tensor("psa", [P, tt], F32))
        psb = ctx.enter_context(nc.sbuf_tensor("psb", [P, tt], F32))
        ps = ctx.enter_context(nc.sbuf_tensor("ps", [P, tt], F32))
        dr1 = ctx.enter_context(nc.sbuf_tensor("dr1", [P, 1], F32))
        dr3 = ctx.enter_context(nc.sbuf_tensor("dr3", [P, 1], F32))
        dr2 = ctx.enter_context(nc.sbuf_tensor("dr2", [P, 1], F16))
        rsqh = ctx.enter_context(nc.sbuf_tensor("rsqh", [P, tt], F16))
        rx8 = ctx.enter_context(nc.sbuf_tensor("rx8", [P, tt], F32))
        ptc = [
            ctx.enter_context(nc.sbuf_tensor(f"ptc{k}", [ct, P], F16))
            for k in range(H)
        ]
        cps_a = ctx.enter_context(nc.psum_tensor("cps_a", [P, w // 2], F32))
        pt = [
            ctx.enter_context(nc.psum_tensor(f"pt{k}", [ct, P], F16))
            for k in range(H)
        ]
        cps = [cps_a, cps_a]

        s_in = [ctx.enter_context(nc.semaphore(f"s_in{t}")) for t in range(tt)]
        s_eye = ctx.enter_context(nc.semaphore("s_eye"))
        s_rdv = ctx.enter_context(nc.semaphore("s_rdv"))  # DVE reduce count
        s_redA = ctx.enter_context(nc.semaphore("s_redA"))  # raw ACT accum count
        s_redAd = [ctx.enter_context(nc.semaphore(f"s_redAd{k}")) for k in range(H)]
        s_ps = [ctx.enter_context(nc.semaphore(f"s_ps{k}")) for k in range(H)]
        s_sq = ctx.enter_context(nc.semaphore("s_sq"))  # raw sqrt count
        s_sqd = [ctx.enter_context(nc.semaphore(f"s_sqd{k}")) for k in range(H)]
        s_tpl = [ctx.enter_context(nc.semaphore(f"s_tpl{k}")) for k in range(H)]
        s_ptc = [ctx.enter_context(nc.semaphore(f"s_ptc{k}")) for k in range(H)]
        s_ccin = [ctx.enter_context(nc.semaphore(f"s_ccin{k}")) for k in range(H)]
        s_cc = [ctx.enter_context(nc.semaphore(f"s_cc{k}")) for k in range(H)]
        s_ccrow = [
            ctx.enter_context(nc.semaphore(f"s_ccrow{k}")) for k in range(H)
        ]
        s_bc = [ctx.enter_context(nc.semaphore(f"s_bc{k}")) for k in range(H)]
        s_pscp = ctx.enter_context(nc.semaphore("s_pscp"))
        s_tsr = ctx.enter_context(nc.semaphore("s_tsr"))  # raw ACT row-scale
        s_ts = [ctx.enter_context(nc.semaphore(f"s_ts{k}")) for k in range(H)]
        s_stt = ctx.enter_context(nc.semaphore("s_stt"))
        s_souts = ctx.enter_context(nc.semaphore("s_souts"))
        block = ctx.enter_context(nc.Block())

        def accols(t):
            return ACC if t % ct == ct - 1 else AC

        # permuted view of the raw-broadcast AG output for chunk k:
        # raw index (c, t2, p); global column order within the half is
        # (t2, c, p). Inner 128 runs stay contiguous.
        def colscale_perm(k):
            return colscale[:, k * w : (k + 1) * w].rearrange(
                "q (c t p) -> q t c p", c=ncores, t=ct, p=P
            )

        def tile_half_v(t, k):
            return tiles[t][:, k * w : (k + 1) * w].rearrange(
                "q (t c p) -> q t c p", t=ct, c=ncores, p=P
            )

        @block.gpsimd
        def _(g):
            for t in range(tt):
                g.dma_start(tiles[t][:, :], mx_l[t, :, :]).then_inc(s_in[t], 16)
            # doorbell AG0 only once the loads are done: the ncfw entry
            # phase stalls under heavy HBM traffic anyway, and a post-load
            # doorbell measures ~3 us faster to ring-start
            g.wait_ge(s_in[tt - 1], 16)
            for k in range(H):
                g.wait_ge(s_ccin[k], 16)
                g.collective_compute(
                    "AllGather",
                    mybir.AluOpType.bypass,
                    replica_groups=[list(range(ncores))],
                    ins=[cc_in[k][:]],
                    outs=[cc_out[k][:]],
                ).then_inc(s_cc[k], 1)

        @block.sync
        def _(sp):
            sp.dma_start(eye_sb[:, :], eye[:, :]).then_inc(s_eye, 16)
            sp.dma_start(ones_sb[:, :], ones[:, :]).then_inc(s_eye, 16)
            # AG payloads (1 KiB each) on the sync HWDGE ring, ahead of the
            # stores in program order
            for k in range(H):
                sp.wait_ge(s_ptc[k], 1)
                sp.dma_start(cc_in[k][:], ptc[k][:, :]).then_inc(s_ccin[k], 16)
            # gathered r_inv row for chunk 0 -> partition 0 (8 KiB); the
            # [128 x 4096] broadcast happens on the PE (ones @ row into
            # PSUM) + a DVE copy - the DRE partition-broadcast DMA path
            # measures 8-14 us for the same result
            sp.wait_ge(s_cc[0], 1)
            sp.dma_start(ccrow[0][:, :], cc_out[0][:]).then_inc(s_ccrow[0], 16)
            # half-0 stores on the sync ring (half-1 goes on the ACT ring so
            # the two HWDGE rings drain concurrently; splitting half-1
            # across both rings measured slower - 156 vs 147 us)
            for t in range(tt):
                sp.wait_ge(s_stt, t + 1)
                sp.dma_start(out_v[t, :, 0], tiles[t][:, 0:w]).then_inc(
                    s_souts, 16
                )
            sp.wait_ge(s_souts, 16 * tt * H)

        @block.scalar
        def _(s):
            for k in range(H):
                # rowsum prefix per tile: in-place Copy with f32 accum
                # (source offset 0 keeps the 2x perf mode)
                for t in range(k * ct, (k + 1) * ct):
                    s.wait_ge(s_in[t], 16)
                    s.activation(
                        tiles[t][:, 0 : accols(t)],
                        tiles[t][:, 0 : accols(t)],
                        mybir.ActivationFunctionType.Copy,
                        accum_out=psa[:, t : t + 1],
                    ).then_inc(s_redA, 1)
                # drain own accum writebacks (self-wait), then publish
                s.wait_ge(s_redA, (k + 1) * ct)
                s.activation(
                    dr1[:, :],
                    psa[:, (k + 1) * ct - 1 : (k + 1) * ct],
                    mybir.ActivationFunctionType.Copy,
                ).then_inc(s_redAd[k], 1)
                # rsq' = sqrt(rowsum * 2^-26)  (fp16 value ~0.7)
                s.wait_ge(s_ps[k], 1)
                s.activation(
                    rsqh[:, k * ct : (k + 1) * ct],
                    ps[:, k * ct : (k + 1) * ct],
                    mybir.ActivationFunctionType.Sqrt,
                    scale=1.0 / SCALE_OUT,
                ).then_inc(s_sq, 1)
                # drain + publish rsqh for PE/DVE readers
                s.wait_ge(s_sq, k + 1)
                s.activation(
                    dr2[:, :],
                    rsqh[:, (k + 1) * ct - 1 : (k + 1) * ct],
                    mybir.ActivationFunctionType.Copy,
                ).then_inc(s_sqd[k], 1)
            # row scale for tiles 4..7 on ACT while it is otherwise idle in
            # the AG1 window; h-half-major so DVE's half-0 TTs are gated
            # only on the first four. s_ptc[1] implies rx8[:, 4:8] is ready
            # (DVE computes rx8 chunk 1 before the ptc1 reciprocal).
            s.wait_ge(s_ptc[1], 1)
            for h in range(H):
                for t in range(ct, tt):
                    s.activation(
                        tiles[t][:, h * w : (h + 1) * w],
                        tiles[t][:, h * w : (h + 1) * w],
                        mybir.ActivationFunctionType.Copy,
                        scale=rx8[:, t : t + 1],
                    ).then_inc(s_tsr, 1)
                # drain + publish the half's row scales
                s.wait_ge(s_tsr, (h + 1) * ct)
                s.activation(
                    dr2[:, :],
                    rsqh[:, tt - 1 : tt],
                    mybir.ActivationFunctionType.Copy,
                ).then_inc(s_ts[h], 1)
            # gathered r_inv row for chunk 1 -> partition 0 (ACT ring)
            # (moving the chunk-1 PSUM->SBUF colscale copies here onto ACT
            # measured SLOWER overall: 153.8 vs 145.8 us - ACT runs the
            # fp32-PSUM Copy at 1x and the h0->h1 overlap win didn't pay)
            s.wait_ge(s_cc[1], 1)
            s.dma_start(ccrow[1][:, :], cc_out[1][:]).then_inc(s_ccrow[1], 16)
            # half-1 stores on the ACT ring, concurrent with sync's half-0
            for t in range(tt):
                s.wait_ge(s_stt, tt + t + 1)
                s.dma_start(out_v[t, :, 1], tiles[t][:, w:n]).then_inc(
                    s_souts, 16
                )

        @block.tensor
        def _(pe):
            pe.wait_ge(s_eye, 32)
            for k in range(H):
                pe.wait_ge(s_sqd[k], 1)
                pe.transpose(
                    pt[k][:, :], rsqh[:, k * ct : (k + 1) * ct], eye_sb[:, :]
                ).then_inc(s_tpl[k], 1)
            # colscale broadcast: ones[1,128].T @ ccrow[1,...] -> fp32 PSUM
            # in 2048-col pieces (4 x 512-col bank matmuls each), ping-pong
            # with the DVE fp32->fp16 copies via s_pscp
            # cps_a aliases the transpose outputs: both ptc reciprocals must
            # be done before the first broadcast matmul overwrites them
            pe.wait_ge(s_ptc[1], 1)
            m = 0
            for k in range(H):
                pe.wait_ge(s_ccrow[k], 16)
                for j in range(2):
                    if m >= 1:
                        # WAR: the previous piece must have left the buffer
                        pe.wait_ge(s_pscp, m)
                    for i in range(4):
                        mm = pe.matmul(
                            cps[j][:, i * 512 : (i + 1) * 512],
                            ones_sb[:, :],
                            ccrow[k][:, j * 2048 + i * 512 : j * 2048 + (i + 1) * 512],
                            start=True,
                            stop=True,
                        )
                        if i == 3:
                            mm.then_inc(s_bc[k], 1)
                    m += 1

        @block.vector
        def _(v):
            def red(t):
                v.wait_ge(s_in[t], 16)
                v.tensor_reduce(
                    psb[:, t : t + 1],
                    tiles[t][:, accols(t) : n],
                    mybir.AxisListType.X,
                    mybir.AluOpType.add,
                ).then_inc(s_rdv, 1)

            def chunk_chain(k):
                # tensor_reduce writebacks are lazy (accumulation path, like
                # ACT accum_out): self-wait until the closing reduce retires,
                # then a dummy read, before combining - otherwise the comb
                # can read a stale psb (measured: tiles 7 on 2 of 8 cores
                # lost the whole DVE partial when ACT was not the laggard)
                v.wait_ge(s_rdv, (k + 1) * ct)
                v.tensor_scalar_add(
                    dr3[:, :], psb[:, (k + 1) * ct - 1 : (k + 1) * ct], 0.0
                )
                # combine rowsum halves (psa safe after ACT drain-publish)
                v.wait_ge(s_redAd[k], 1)
                v.tensor_tensor(
                    ps[:, k * ct : (k + 1) * ct],
                    psa[:, k * ct : (k + 1) * ct],
                    psb[:, k * ct : (k + 1) * ct],
                    mybir.AluOpType.add,
                ).then_inc(s_ps[k], 1)
                # row-scale scalars (f32) while the PE transpose runs; must
                # precede the ptc reciprocal: ACT's tiles-4..7 row scale is
                # gated on s_ptc[1] and reads rx8[:, 4:8]
                v.wait_ge(s_sqd[k], 1)
                v.reciprocal(
                    rx8[:, k * ct : (k + 1) * ct], rsqh[:, k * ct : (k + 1) * ct]
                )
                with nc.allow_low_precision(reason="fp16 r_inv, tol 2e-2"):
                    # transposed reciprocal straight out of PSUM -> AG payload
                    v.wait_ge(s_tpl[k], 1)
                    v.reciprocal(ptc[k][:, :], pt[k][:, :]).then_inc(s_ptc[k], 1)

            def rowscale(t):
                for hh in range(H):
                    v.tensor_scalar_mul(
                        tiles[t][:, hh * w : (hh + 1) * w],
                        tiles[t][:, hh * w : (hh + 1) * w],
                        rx8[:, t : t + 1],
                    )

            # chunk 0: reduces + r_inv chain -> AG0 doorbell ~47 us
            for t in range(ct):
                red(t)
            chunk_chain(0)
            # chunk 1 reduces, interleaved with chunk-0 row scales; the
            # closing reduce stays unobstructed so AG1 fires on time
            red(ct)
            rowscale(0)
            red(ct + 1)
            rowscale(1)
            red(ct + 2)
            rowscale(2)
            red(ct + 3)
            chunk_chain(1)
            rowscale(3)
            # column scale, half 0 then half 1; tiles 4-7's row scale runs
            # on ACT, gated per half via s_ts; colscale lands via PE matmul
            # broadcast into PSUM + one DVE copy per half
            def cscopy(h, j):
                # PSUM fp32 -> SBUF fp16 colscale piece; the s_pscp incs
                # stay in global piece order so the PE WAR chain holds
                v.wait_ge(s_bc[h], j + 1)
                v.tensor_scalar_add(
                    colscale[:, h * w + j * 2048 : h * w + (j + 1) * 2048],
                    cps[j][:, :],
                    0.0,
                ).then_inc(s_pscp, 1)

            def cstt(t, h):
                if t == ct:
                    v.wait_ge(s_ts[h], 1)
                v.tensor_tensor(
                    tile_half_v(t, h),
                    tile_half_v(t, h),
                    colscale_perm(h),
                    mybir.AluOpType.mult,
                ).then_inc(s_stt, 1)

            cscopy(0, 0)
            cscopy(0, 1)
            # the chunk-1 copies slide INSIDE the half-0 TT stream so the
            # PE's single-PSUM-buffer refill turnaround hides under TTs
            # (copies at the h0->h1 boundary measured a 2.1 us stall)
            for t in range(ct + 1):
                cstt(t, 0)
            cscopy(1, 0)
            for t in range(ct + 1, tt):
                cstt(t, 0)
            cscopy(1, 1)
            for t in range(tt):
                cstt(t, 1)

    return nc


_NC_CACHE = {}


def _get_nc(n=N, ncores=NCORES):
    key = (n, ncores)
    if key not in _NC_CACHE:
        _NC_CACHE[key] = build_kernel(n, ncores)
    return _NC_CACHE[key]


def kernel(adj, **run_kwargs):
    adj = np.asarray(adj)
    assert adj.shape == (N, N) and adj.dtype == np.float32
    mxh = (adj * SCALE_IN).astype(np.float16)
    idx = np.arange(N)
    mxh[idx, idx] = (
        adj[idx, idx].astype(np.float64) * SCALE_IN + SCALE_IN
    ).astype(np.float16)
    eye = np.eye(P, dtype=np.float16)
    ones = np.ones((1, P), dtype=np.float16)

    # interleaved sharding: core c's tile t = global rows t*1024 + c*128
    mxv = mxh.reshape(T, NCORES, P, N)
    in_maps = [
        {
            "mx": np.ascontiguousarray(mxv[:, c]).reshape(SHARD, N),
            "eye": eye,
            "ones": ones,
        }
        for c in range(NCORES)
    ]
    nc = _get_nc()
    try:
        res = run_bass_kernel_spmd(nc, in_maps, list(range(NCORES)), **run_kwargs)
    except Exception:
        import time

        time.sleep(2.0)
        res = run_bass_kernel_spmd(nc, in_maps, list(range(NCORES)), **run_kwargs)

    full = np.empty((T, NCORES, P, N), dtype=np.float32)
    for c in range(NCORES):
        full[:, c] = (
            res.results[c]["out"].astype(np.float32).reshape(T, P, N)
        )
    full = full.reshape(N, N) / SCALE_OUT
    if run_kwargs:
        return full, res
    return full
